# revision 1
# baseline (speedup 1.0000x reference)
"""Trainium2 Bass kernel for nn_Decoder (3-layer GRU freerun decoder, argmax feedback).

Model: T=128 steps, B=512, H=1024, V=64, L=3, input = [latent(256) | onehot(64) | enthalpy(1)].

Sharding: model-parallel over hidden units. Each of the 8 cores owns a 128-unit
slice of every layer (weight rows for its r/z/n gates), computes its gate slice
for the FULL batch (B=512 rides the matmul moving dim), and the per-layer hidden
slices are AllGathered each step. The fc logits are computed as per-core partials
carried inside the layer-2 AllGather, then summed in a fixed order on every core
(bit-identical across ranks) so the argmax trajectories stay in lockstep.

All matmuls run in true fp32 (4 cycles/row on the PE) — the argmax feedback makes
the output chaotic (min top-2 logit gap ~2.6e-7), so fp22/bf16 shortcuts flip
argmaxes and cascade. The latent contribution to layer-0's input gates is constant
across steps and is precomputed on the host.
"""
import os
import sys

sys.path.insert(0, "/opt/trn_rl_repo")

import numpy as np
from concourse import bacc, tile, mybir
from concourse.bass_utils import run_bass_kernel_spmd

NCORES = 8
T = int(os.environ.get("KERNEL_T", "128"))
B, H, V, L, LAT = 512, 1024, 64, 3, 256
HC = H // NCORES          # 128 hidden units per core
KT = H // 128             # 8 k-tiles over the hidden dim
NB = B // 128             # 4 batch chunks of 128
F32 = mybir.dt.float32
AF = mybir.ActivationFunctionType
ALU = mybir.AluOpType


def build_program():
    nc = bacc.Bacc(None, target_bir_lowering=False, num_devices=NCORES)

    # ---- per-core inputs ----
    d_whh = [nc.declare_dram_parameter(f"whh{l}", [128, KT * 384], F32, isOutput=False) for l in range(L)]
    d_wih = {l: nc.declare_dram_parameter(f"wih{l}", [128, KT * 384], F32, isOutput=False) for l in (1, 2)}
    d_woh = nc.declare_dram_parameter("woh", [64, 384], F32, isOutput=False)
    d_glat0 = nc.declare_dram_parameter("glat0", [128, 3 * 512], F32, isOutput=False)
    d_glat = nc.declare_dram_parameter("glat", [128, 3 * 512], F32, isOutput=False)
    d_bias = nc.declare_dram_parameter("bias", [128, 4 * L], F32, isOutput=False)
    d_fcw = nc.declare_dram_parameter("fcw", [128, 64], F32, isOutput=False)
    d_fcb = nc.declare_dram_parameter("fcb", [128, 256], F32, isOutput=False)
    d_iorev = nc.declare_dram_parameter("iorev", [128, 64], F32, isOutput=False)
    d_ident = nc.declare_dram_parameter("ident", [128, 128], F32, isOutput=False)
    d_out = nc.declare_dram_parameter("out", [B, T, V], F32, isOutput=True)

    rg = [list(range(NCORES))]

    with tile.TileContext(nc) as tc:
        with (
            tc.tile_pool(name="const", bufs=1) as cp,
            tc.tile_pool(name="state", bufs=1) as sp,
            tc.tile_pool(name="work", bufs=2) as wp,
            tc.tile_pool(name="psum", bufs=2, space="PSUM") as pp,
            tc.tile_pool(name="dram", bufs=2, space="DRAM") as dp,
        ):
            # ---- load constants ----
            whh = [cp.tile([128, KT, 384], F32, name=f"whh{l}_t") for l in range(L)]
            wih = {l: cp.tile([128, KT, 384], F32, name=f"wih{l}_t") for l in (1, 2)}
            for l in range(L):
                nc.sync.dma_start(out=whh[l][:], in_=d_whh[l][:].rearrange("p (k m) -> p k m", k=KT))
            for l in (1, 2):
                nc.sync.dma_start(out=wih[l][:], in_=d_wih[l][:].rearrange("p (k m) -> p k m", k=KT))
            woh = cp.tile([64, 384], F32)
            glat0 = cp.tile([128, 3, 512], F32)
            glat = cp.tile([128, 3, 512], F32)
            bias = cp.tile([128, 4 * L], F32)
            fcw = cp.tile([128, 64], F32)
            fcb = cp.tile([128, 256], F32)
            iorev = cp.tile([128, 64], F32)
            ident = cp.tile([128, 128], F32)
            nc.sync.dma_start(out=woh[:], in_=d_woh[:])
            nc.sync.dma_start(out=glat0[:], in_=d_glat0[:].rearrange("p (g f) -> p g f", g=3))
            nc.sync.dma_start(out=glat[:], in_=d_glat[:].rearrange("p (g f) -> p g f", g=3))
            nc.sync.dma_start(out=bias[:], in_=d_bias[:])
            nc.sync.dma_start(out=fcw[:], in_=d_fcw[:])
            nc.sync.dma_start(out=fcb[:], in_=d_fcb[:])
            nc.sync.dma_start(out=iorev[:], in_=d_iorev[:])
            nc.sync.dma_start(out=ident[:], in_=d_ident[:])

            # ---- persistent state ----
            # full gathered hiddens; layer 2's carries the fc partials too
            h0f = sp.tile([128, KT, 512], F32)
            h1f = sp.tile([128, KT, 512], F32)
            h2f = sp.tile([128, KT, 768], F32)
            nc.vector.memset(h0f[:], 0.0)
            nc.vector.memset(h1f[:], 0.0)
            nc.vector.memset(h2f[:], 0.0)

            hfull = [h0f, h1f, h2f]
            hc_prev = []
            for l in range(L):
                t0 = wp.tile([128, 512], F32, tag=f"hc{l}", name=f"hc{l}_init")
                nc.vector.memset(t0[:], 0.0)
                hc_prev.append(t0)

            iorev_b = iorev[:].rearrange("p (a v) -> p a v", a=1).to_broadcast([128, NB, 64])

            def bias_ap(l, j):
                return bias[:, 4 * l + j : 4 * l + j + 1]

            # state carried between steps at trace time
            xoh_pending = None  # (onehot_tile,) from previous step, to transpose
            xoh = None          # [64, 512] transposed onehot for the oh-matmuls

            def gate_psums(l, t, xoh_local):
                """Emit the recurrent (gh) matmul groups for layer l. Returns psums.
                The gx / onehot portions are appended by the caller (they close the
                groups), so r/z groups stay open (stop=False) unless layer 0 at t=0.
                """
                wh = whh[l]
                hf = hfull[l]
                ps = {}
                for gi, gname in enumerate(("r", "z")):
                    p = pp.tile([128, 512], F32, tag=f"p{gname}", name=f"ps_{gname}_{l}_{t}")
                    for k in range(KT):
                        last = (l == 0 and t == 0 and k == KT - 1)
                        nc.tensor.matmul(
                            p[:], wh[:, k, gi * 128 : (gi + 1) * 128],
                            hf[:, k, 0:512], start=(k == 0), stop=last,
                        )
                    ps[gname] = p
                p = pp.tile([128, 512], F32, tag="phn", name=f"ps_hn_{l}_{t}")
                for k in range(KT):
                    nc.tensor.matmul(
                        p[:], wh[:, k, 256:384], hf[:, k, 0:512],
                        start=(k == 0), stop=(k == KT - 1),
                    )
                ps["hn"] = p
                return ps

            def close_l0(ps, t, xoh_local):
                """Append onehot matmuls closing layer-0 groups (t>=1)."""
                for gi, gname in enumerate(("r", "z")):
                    nc.tensor.matmul(
                        ps[gname][:], woh[:, gi * 128 : (gi + 1) * 128], xoh_local[:],
                        start=False, stop=True,
                    )
                p = pp.tile([128, 512], F32, tag="pxn", name=f"ps_xn_0_{t}")
                nc.tensor.matmul(p[:], woh[:, 256:384], xoh_local[:], start=True, stop=True)
                ps["xn"] = p

            def close_l12(ps, l, t):
                """Append the gx matmuls (from previous layer's gathered h) for l=1,2."""
                wi = wih[l]
                hf = hfull[l - 1]
                for gi, gname in enumerate(("r", "z")):
                    for k in range(KT):
                        nc.tensor.matmul(
                            ps[gname][:], wi[:, k, gi * 128 : (gi + 1) * 128],
                            hf[:, k, 0:512], start=False, stop=(k == KT - 1),
                        )
                p = pp.tile([128, 512], F32, tag="pxn", name=f"ps_xn_{l}_{t}")
                for k in range(KT):
                    nc.tensor.matmul(
                        p[:], wi[:, k, 256:384], hf[:, k, 0:512],
                        start=(k == 0), stop=(k == KT - 1),
                    )
                ps["xn"] = p

            def elementwise(l, t, ps):
                """Gate math + hidden update for layer l. Returns new hidden slice."""
                g0 = glat0 if (l == 0 and t == 0) else glat
                if l == 0:
                    tr_ = wp.tile([128, 512], F32, tag="tmpa", name=f"tr_{l}_{t}")
                    nc.vector.tensor_add(tr_[:], ps["r"][:], g0[:, 0, :])
                    r_in = tr_
                    tz_ = wp.tile([128, 512], F32, tag="tmpb", name=f"tz_{l}_{t}")
                    nc.vector.tensor_add(tz_[:], ps["z"][:], g0[:, 1, :])
                    z_in = tz_
                else:
                    r_in, z_in = ps["r"], ps["z"]
                r = wp.tile([128, 512], F32, tag="r", name=f"r_{l}_{t}")
                nc.scalar.activation(r[:], r_in[:], AF.Sigmoid, bias=bias_ap(l, 0))
                z = wp.tile([128, 512], F32, tag="z", name=f"z_{l}_{t}")
                nc.scalar.activation(z[:], z_in[:], AF.Sigmoid, bias=bias_ap(l, 1))
                # n = tanh(gxn + b_ihn + r*(ghn + b_hhn))
                t1 = wp.tile([128, 512], F32, tag="t1", name=f"t1_{l}_{t}")
                nc.vector.scalar_tensor_tensor(
                    out=t1[:], in0=ps["hn"][:], scalar=bias_ap(l, 2), in1=r[:],
                    op0=ALU.add, op1=ALU.mult,
                )
                t2 = wp.tile([128, 512], F32, tag="t2", name=f"t2_{l}_{t}")
                if "xn" in ps:
                    nc.vector.tensor_add(t2[:], t1[:], ps["xn"][:])
                    t_pre = t2
                else:
                    t_pre = t1
                if l == 0:
                    t3 = wp.tile([128, 512], F32, tag="t3", name=f"t3_{l}_{t}")
                    nc.vector.tensor_add(t3[:], t_pre[:], g0[:, 2, :])
                    t_pre = t3
                n = wp.tile([128, 512], F32, tag="n", name=f"n_{l}_{t}")
                nc.scalar.activation(n[:], t_pre[:], AF.Tanh, bias=bias_ap(l, 3))
                # h' = n + z*(h_prev - n)
                d = wp.tile([128, 512], F32, tag="d", name=f"d_{l}_{t}")
                nc.vector.tensor_sub(d[:], hc_prev[l][:], n[:])
                e = wp.tile([128, 512], F32, tag="e", name=f"e_{l}_{t}")
                nc.vector.tensor_mul(e[:], z[:], d[:])
                hc = wp.tile([128, 512], F32, tag=f"hc{l}", name=f"hc{l}_{t}")
                nc.vector.tensor_add(hc[:], n[:], e[:])
                return hc

            def allgather_h(l, t, hc, fc_sb=None):
                w = 768 if l == 2 else 512
                gin = dp.tile([128, w], F32, name=f"agin{l}_{t}", tag=f"agin{l}")
                nc.sync.dma_start(out=gin[:, 0:512], in_=hc[:])
                if l == 2:
                    nc.sync.dma_start(out=gin[:, 512:768], in_=fc_sb[:])
                gout = dp.tile([NCORES * 128, w], F32, name=f"agout{l}_{t}", tag=f"agout{l}",
                               addr_space="Shared")
                nc.gpsimd.collective_compute(
                    "AllGather", ALU.bypass, replica_groups=rg,
                    ins=[gin[:]], outs=[gout[:]],
                )
                nc.sync.dma_start(
                    out=hfull[l][:],
                    in_=gout[:].rearrange("(r p) f -> p r f", p=128),
                )

            for t in range(T):
                # ---- layer 0: recurrent matmuls first ----
                ps0 = gate_psums(0, t, xoh)

                # transpose previous step's onehot now — its argmax chain has had
                # a full gh-block of PE time to complete, so no PE stall here
                if xoh_pending is not None:
                    oh_prev = xoh_pending
                    ps_tr = pp.tile([64, 512], F32, tag="pxn", name=f"ps_tr_{t}")
                    for j in range(NB):
                        nc.tensor.transpose(ps_tr[:, j * 128 : (j + 1) * 128], oh_prev[:, j, :], ident[:])
                    xoh = wp.tile([64, 512], F32, tag="xoh", name=f"xoh_{t}")
                    nc.vector.tensor_copy(xoh[:], ps_tr[:])
                    xoh_pending = None
                    close_l0(ps0, t, xoh)

                hc0 = elementwise(0, t, ps0)
                hc_prev[0] = hc0
                allgather_h(0, t, hc0)

                # ---- layer 1 ----
                ps1 = gate_psums(1, t, None)
                close_l12(ps1, 1, t)
                hc1 = elementwise(1, t, ps1)
                hc_prev[1] = hc1
                allgather_h(1, t, hc1)

                # ---- layer 2 ----
                ps2 = gate_psums(2, t, None)
                close_l12(ps2, 2, t)
                hc2 = elementwise(2, t, ps2)
                hc_prev[2] = hc2

                # fc partials: [128b, 64] per batch chunk
                ps_fc = pp.tile([128, 256], F32, tag="pxn", name=f"ps_fc_{t}")
                for j in range(NB):
                    nc.tensor.matmul(
                        ps_fc[:, j * 64 : (j + 1) * 64],
                        hc2[:, j * 128 : (j + 1) * 128], fcw[:],
                        start=True, stop=True,
                    )
                fc_sb = wp.tile([128, 256], F32, tag="fcsb", name=f"fcsb_{t}")
                nc.vector.tensor_copy(fc_sb[:], ps_fc[:])
                allgather_h(2, t, hc2, fc_sb)

                # ---- logits = fixed-order sum of the 8 gathered partials + bias ----
                lg = wp.tile([128, 256], F32, tag="lg", name=f"lg_{t}")
                nc.vector.tensor_add(lg[:], h2f[:, 0, 512:768], h2f[:, 1, 512:768])
                for rr in range(2, NCORES):
                    nc.vector.tensor_add(lg[:], lg[:], h2f[:, rr, 512:768])
                nc.vector.tensor_add(lg[:], lg[:], fcb[:])

                # ---- first-index argmax one-hot over each 64-wide block ----
                lv = lg[:].rearrange("p (a v) -> p a v", a=NB)
                mx = wp.tile([128, NB], F32, tag="mx", name=f"mx_{t}")
                nc.vector.tensor_reduce(out=mx[:], in_=lv, op=ALU.max, axis=mybir.AxisListType.X)
                oh0 = wp.tile([128, NB, 64], F32, tag="oh0", name=f"oh0_{t}")
                nc.vector.tensor_tensor(out=oh0[:], in0=lv, in1=mx[:].to_broadcast([128, NB, 64]), op=ALU.is_equal)
                m1 = wp.tile([128, NB, 64], F32, tag="m1", name=f"m1_{t}")
                nc.vector.tensor_tensor(out=m1[:], in0=oh0[:], in1=iorev_b, op=ALU.mult)
                srev = wp.tile([128, NB], F32, tag="srev", name=f"srev_{t}")
                nc.vector.tensor_reduce(out=srev[:], in_=m1[:], op=ALU.max, axis=mybir.AxisListType.X)
                oh = wp.tile([128, NB, 64], F32, tag="oh", name=f"oh_{t}")
                nc.vector.tensor_tensor(out=oh[:], in0=m1[:], in1=srev[:].to_broadcast([128, NB, 64]), op=ALU.is_equal)

                for j in range(NB):
                    nc.sync.dma_start(out=d_out[j * 128 : (j + 1) * 128, t, :], in_=oh[:, j, :])
                xoh_pending = oh

    nc.finalize()
    return nc


_program = None


def _get_program():
    global _program
    if _program is None:
        _program = build_program()
    return _program


def _prep_core_inputs(latent, enth, params):
    """Host-side per-core weight slicing / constant precompute (fp64 where free)."""
    lat64 = latent.astype(np.float64)
    enth64 = enth.astype(np.float64)
    layers = params["layers"]
    fc_w = np.asarray(params["fc_w"], np.float32)
    fc_b = np.asarray(params["fc_b"], np.float32)

    iorev = np.tile((64.0 - np.arange(64, dtype=np.float32))[None, :], (128, 1))
    ident = np.eye(128, dtype=np.float32)
    fcb = np.tile(fc_b[None, :], (128, 4)).astype(np.float32)

    maps = []
    for c in range(NCORES):
        u0 = c * HC
        m = {}
        for l in range(L):
            w_ih = np.asarray(layers[l]["w_ih"], np.float32)
            w_hh = np.asarray(layers[l]["w_hh"], np.float32)
            b_ih = np.asarray(layers[l]["b_ih"], np.float32)
            b_hh = np.asarray(layers[l]["b_hh"], np.float32)
            rows = np.concatenate([
                np.arange(u0, u0 + HC),
                np.arange(H + u0, H + u0 + HC),
                np.arange(2 * H + u0, 2 * H + u0 + HC),
            ])
            whh_s = w_hh[rows]                       # [384, 1024]
            m[f"whh{l}"] = np.ascontiguousarray(
                whh_s.T.reshape(KT, 128, 384).transpose(1, 0, 2).reshape(128, KT * 384)
            )
            wih_s = w_ih[rows]                       # [384, in_dim]
            if l == 0:
                w_lat = wih_s[:, 0:LAT]              # [384, 256]
                w_oh = wih_s[:, LAT:LAT + V]         # [384, 64]
                w_en = wih_s[:, LAT + V]             # [384]
                m["woh"] = np.ascontiguousarray(w_oh.T)
                g = lat64 @ w_lat.astype(np.float64).T          # [512, 384]
                g0 = g + enth64[:, None] * w_en.astype(np.float64)[None, :]
                for nm, arr in (("glat", g), ("glat0", g0)):
                    a32 = arr.astype(np.float32).T               # [384, 512]
                    m[nm] = np.ascontiguousarray(
                        a32.reshape(3, 128, 512).transpose(1, 0, 2).reshape(128, 3 * 512)
                    )
            else:
                m[f"wih{l}"] = np.ascontiguousarray(
                    wih_s.T.reshape(KT, 128, 384).transpose(1, 0, 2).reshape(128, KT * 384)
                )
            bcol = np.zeros((128, 4), np.float32)
            bcol[:, 0] = b_ih[rows[:HC]] + b_hh[rows[:HC]]
            bcol[:, 1] = b_ih[rows[HC:2 * HC]] + b_hh[rows[HC:2 * HC]]
            bcol[:, 2] = b_hh[rows[2 * HC:]]
            bcol[:, 3] = b_ih[rows[2 * HC:]]
            m.setdefault("_bias", []).append(bcol)
        m["bias"] = np.ascontiguousarray(np.concatenate(m.pop("_bias"), axis=1))
        m["fcw"] = np.ascontiguousarray(fc_w[:, u0:u0 + HC].T)
        m["fcb"] = fcb
        m["iorev"] = iorev
        m["ident"] = ident
        maps.append(m)
    return maps


def kernel(latent_vec, enthalpy, inp, params, _trace=False):
    latent = np.asarray(latent_vec, np.float32)
    enth = np.asarray(enthalpy, np.float32)
    del inp  # unused on the freerun path
    nc = _get_program()
    in_maps = _prep_core_inputs(latent, enth, params)
    res = run_bass_kernel_spmd(nc, in_maps, core_ids=list(range(NCORES)), trace=_trace)
    out = np.asarray(res.results[0]["out"], np.float32)
    if _trace:
        return out, res
    return out
